# revision 3
# baseline (speedup 1.0000x reference)
"""Adaptive embedding lookup (nn.AdaptiveEmbedding) on 8 TRN2 NeuronCores.

Strategy (data-parallel over tokens, tables replicated, no collectives):

Host:
  - Clusters 0 and 1 are FUSED on host: table' = (emb @ proj.T) * scale in
    bf16 [20000, 1024].  On device those clusters are a pure dma_gather
    (transpose=False, token-on-partition layout) whose destination tile is
    DMA'd straight to the output rows — no projection load, no matmul.
  - Bucket the 16384 tokens by embedding cluster (cluster 2 is split into
    5 sub-ranges of 32000 rows so dma_gather's int16 indices stay in range,
    and cluster 3's table is packed 8-rows-per-256B-super-row), deal each
    bucket's tokens round-robin to the 8 cores, pad each per-core bucket to
    a multiple of 128 (one "group" of 128 output rows).  Fused-cluster pad
    indices are -1 (skipped by the gather ucode -> no wasted bandwidth).
  - emb2 f32->bf16 zero-padded to 256B rows; emb3 packed bf16 [, 128];
    proj2/proj3 pre-transposed, pre-scaled by sqrt(d_proj), bf16.

Device (SPMD, identical graph on all 8 cores, one TileContext):
  - Fused clusters: per-group dma_gather [128, 1, 1024]bf16 -> trimmed
    dma_start to the output rows.
  - Cluster 2/3: transposed dma_gather into lhsT layout, (cluster 3:
    mask-select the 16-elem sub-row inside the 128-elem super-row), matmul
    against projection chunks accumulating in PSUM, copy (f32->bf16 cast)
    to SBUF, DMA the trimmed [rows, 1024] output out.
  - The Tile scheduler overlaps gathers, PE work, copies and output DMAs.

Host: inverse-permute the 8 per-core outputs into [8, 2048, 1024] f32.
"""

import numpy as np
import ml_dtypes

import concourse.bacc as bacc
import concourse.bass as bass
import concourse.mybir as mybir
from concourse.bass_utils import run_bass_kernel_spmd
from concourse.tile import TileContext

N_TOKEN = 267735
D_PROJ = 1024
CUTOFF_ENDS = [0, 20000, 40000, 200000, 267735]
D_EMBS = [1024, 256, 64, 16]
EMB_SCALE = float(D_PROJ) ** 0.5
N_CORES = 8
P = 128
NFREE = 512          # psum free-dim per matmul
C2_SUB = 32000       # cluster-2 subtable rows (int16 range)
C2_NSUB = 5
C3_PACK = 8          # cluster-3 rows packed per super-row
C3_SROWS = -(-(CUTOFF_ENDS[4] - CUTOFF_ENDS[3]) // C3_PACK)  # 8467

BF16 = ml_dtypes.bfloat16

# Test-harness knobs (the grader never touches these).
TRACE = False
TRACE_CORES = None
LAST = {}

_GRAPH_CACHE = {}

# unit = gather bucket: 0, 1 (fused), (2, r) for sub-range r, 3.
UNIT_KEYS = [0, 1] + [(2, r) for r in range(C2_NSUB)] + [3]
FUSED_UNITS = (0, 1)


def _build_graph(Ks, rows_g):
    """Ks: dict unit_key -> group count (0 allowed); rows_g: global group ->
    output rows actually used (<=128, pad rows trimmed from the out DMA).
    Same on all cores."""
    key = (tuple(Ks[u] for u in UNIT_KEYS), tuple(sorted(rows_g.items())))
    if key in _GRAPH_CACHE:
        return _GRAPH_CACHE[key]

    K3 = Ks[3]
    NI = 8 * sum(Ks.values())          # idx16 columns (8 per group)
    G = sum(Ks.values())               # total output groups
    NAUX = max(K3, 1) * 128            # c3 sub-row select mask (transposed)

    nc = bacc.Bacc("TRN2", debug=False, num_swdge_queues=4)
    idx_ext = nc.declare_dram_parameter("idx16", [P, max(NI, 16)], mybir.dt.int16, False)
    fe0_ext = nc.declare_dram_parameter("fe0", [20000, D_PROJ], mybir.dt.bfloat16, False)
    fe1_ext = nc.declare_dram_parameter("fe1", [20000, D_PROJ], mybir.dt.bfloat16, False)
    emb2_ext = nc.declare_dram_parameter("emb2p", [C2_SUB * C2_NSUB, 128], mybir.dt.bfloat16, False)
    emb3_ext = nc.declare_dram_parameter("emb3p", [C3_SROWS, 128], mybir.dt.bfloat16, False)
    pt2_ext = nc.declare_dram_parameter("pt2", [64, 1, D_PROJ], mybir.dt.bfloat16, False)
    pt3_ext = nc.declare_dram_parameter("pt3s", [128, 1, D_PROJ], mybir.dt.bfloat16, False)
    aux_ext = nc.declare_dram_parameter("aux", [P, NAUX], mybir.dt.bfloat16, False)
    out_ext = nc.declare_dram_parameter("out", [G * P, D_PROJ], mybir.dt.bfloat16, True)

    with TileContext(nc) as tc:
        with tc.tile_pool(name="const", bufs=1) as constp, \
             tc.tile_pool(name="work", bufs=8) as workp, \
             tc.tile_pool(name="ps_o", bufs=8, space="PSUM") as psump:
            idx_sb = constp.tile([P, max(NI, 16)], mybir.dt.int16, tag="idx")
            nc.sync.dma_start(out=idx_sb[:], in_=idx_ext[:])
            aux_sb = constp.tile([P, NAUX], mybir.dt.bfloat16, tag="aux")
            nc.sync.dma_start(out=aux_sb[:], in_=aux_ext[:])

            # fused-cluster gather destinations: token-on-partition layout,
            # gathered row m -> partition m%128, group m//128
            f_sb = {
                0: constp.tile([P, max(Ks[0], 1), D_PROJ], mybir.dt.bfloat16,
                               tag="f0", name="f0"),
                1: constp.tile([P, max(Ks[1], 1), D_PROJ], mybir.dt.bfloat16,
                               tag="f1", name="f1"),
            }
            fe_ext = {0: fe0_ext, 1: fe1_ext}
            # cluster 2/3 lhsT gather destinations ([dim, .., token] layout):
            # the c2 table is zero-padded to 128-elem bf16 rows and c3's
            # packed super-rows are already 128 wide, so no on-device casts
            # or PE transposes are needed
            e2s = [
                constp.tile([P, 1, max(Ks[(2, r)], 1) * P], mybir.dt.bfloat16,
                            tag=f"e2_{r}", name=f"e2_{r}")
                for r in range(C2_NSUB)
            ]
            e3 = constp.tile([P, 1, max(K3, 1) * P], mybir.dt.bfloat16, tag="e3")

            # idx16 column offset per unit (indices are packed in UNIT_KEYS order)
            unit_col = {}
            col = 0
            for u in UNIT_KEYS:
                unit_col[u] = col
                col += 8 * Ks[u]
            # global group index per unit
            gbase_dev = {}
            acc_g = 0
            for u in UNIT_KEYS:
                gbase_dev[u] = acc_g
                acc_g += Ks[u]

            # gather emission order: fused clusters split per group (their
            # output DMA can start as soon as each group lands), then c3
            # (feeds the mask mult), then cluster-2 sub-gathers
            gather_list = []  # (unit, group_lo, n_groups)
            for glo in range(max(Ks[0], Ks[1])):
                for u in FUSED_UNITS:
                    if glo < Ks[u]:
                        gather_list.append((u, glo, 1))
            for u in [3, (2, 0), (2, 1), (2, 2), (2, 3), (2, 4)]:
                if Ks[u] > 0:
                    gather_list.append((u, 0, Ks[u]))
            for qi, (u, glo, n) in enumerate(gather_list):
                c0 = unit_col[u] + 8 * glo
                if u in FUSED_UNITS:
                    nc.gpsimd.dma_gather(
                        f_sb[u][:, glo:glo + 1, :], fe_ext[u][:],
                        idx_sb[:, c0:c0 + 8 * n], n * P, n * P, D_PROJ,
                        transpose=False,
                        queue_num=qi % 4,
                    )
                    continue
                if u == 3:
                    dst, tab = e3[:], emb3_ext[:]
                else:
                    r = u[1]
                    dst, tab = e2s[r][:], emb2_ext[r * C2_SUB:(r + 1) * C2_SUB, :]
                nc.gpsimd.dma_gather(
                    dst, tab, idx_sb[:, c0:c0 + 8 * n], n * P, n * P, 128,
                    transpose=True,
                    queue_num=qi % 4,
                )

            pt2_sb = constp.tile([64, 1, D_PROJ], mybir.dt.bfloat16, tag="pt2")
            nc.sync.dma_start(out=pt2_sb[:], in_=pt2_ext[:])
            pt3_sb = constp.tile([128, 1, D_PROJ], mybir.dt.bfloat16, tag="pt3")
            nc.scalar.dma_start(out=pt3_sb[:], in_=pt3_ext[:])

            # fused clusters: gather dst -> trimmed output DMA, nothing else
            for u in FUSED_UNITS:
                for j in range(Ks[u]):
                    g = gbase_dev[u] + j
                    rows = rows_g[g]
                    out_eng = nc.sync if g % 2 == 0 else nc.scalar
                    out_eng.dma_start(
                        out=out_ext[g * P:g * P + rows, :],
                        in_=f_sb[u][:rows, j, :],
                    )

            # c3: one fused mask-multiply selecting the 16-elem sub-row of
            # the packed super-row (mask shipped from host in aux,
            # transposed [dim-partition, token-col] layout)
            em = constp.tile([P, max(K3, 1) * P], mybir.dt.bfloat16, tag="em")
            if K3 > 0:
                nc.vector.tensor_tensor(
                    out=em[:], in0=e3[:, 0, :], in1=aux_sb[:, 0:K3 * P],
                    op=mybir.AluOpType.mult,
                )

            # matmul groups (cluster 2 then 3), PE stream pinned in emission
            # order with no-sync scheduling edges
            import bass_rust as _br2
            last_pe_inst = [None]

            def emit_group(pt, d, lhsT_of, g):
                rows = rows_g[g]
                osb = workp.tile([P, D_PROJ], mybir.dt.bfloat16, tag="osb")
                ps0 = psump.tile([P, NFREE], mybir.dt.float32, tag="ps")
                ps1 = psump.tile([P, NFREE], mybir.dt.float32, tag="ps")
                for oc, ps in enumerate((ps0, ps1)):
                    mm = nc.tensor.matmul(
                        out=ps[:],
                        lhsT=lhsT_of(d),
                        rhs=pt[:d, 0, oc * NFREE:(oc + 1) * NFREE],
                        start=True,
                        stop=True,
                    )
                    if last_pe_inst[0] is not None:
                        _br2.add_dep_helper(
                            mm.ins, last_pe_inst[0], sync=False,
                            reason="pin PE stream order",
                        )
                    last_pe_inst[0] = mm.ins
                    nc.any.tensor_copy(
                        out=osb[:, oc * NFREE:(oc + 1) * NFREE], in_=ps[:]
                    )
                out_eng = nc.sync if g % 2 == 0 else nc.scalar
                out_eng.dma_start(
                    out=out_ext[g * P:g * P + rows, :], in_=osb[:rows, :]
                )

            jg = 0
            for r in range(C2_NSUB):
                for j in range(Ks[(2, r)]):
                    gg = gbase_dev[(2, 0)] + jg
                    emit_group(
                        pt2_sb, 64,
                        lambda dk, _r=r, _j=j: e2s[_r][:dk, 0, _j * P:(_j + 1) * P],
                        gg,
                    )
                    jg += 1
            for j in range(K3):
                emit_group(
                    pt3_sb, 128,
                    lambda dk, _j=j: em[:dk, _j * P:(_j + 1) * P],
                    gbase_dev[3] + j,
                )

    nc.compile()
    _GRAPH_CACHE[key] = nc
    return nc


def _wrap_idx16(vals, n_slots, fill=0):
    """int16 values (len <= n_slots, padded with `fill`) -> [128, n_slots/16]
    wrapped."""
    full = np.full(n_slots, fill, dtype=np.int16)
    full[:len(vals)] = vals
    w = np.zeros((16, n_slots // 16), dtype=np.int16)
    m = np.arange(n_slots)
    w[m % 16, m // 16] = full
    return np.tile(w, (8, 1))


def kernel(inp, emb0, emb1, emb2, emb3, proj0, proj1, proj2, proj3):
    inp = np.asarray(inp)
    embs = [np.asarray(e) for e in (emb0, emb1, emb2, emb3)]
    projs = [np.asarray(p) for p in (proj0, proj1, proj2, proj3)]
    B, S = inp.shape
    flat = inp.reshape(-1).astype(np.int64)
    T = flat.shape[0]

    # ---- host-side bucketing -------------------------------------------
    flat = np.clip(flat, 0, N_TOKEN - 1)
    cluster = np.clip(
        np.searchsorted(np.asarray(CUTOFF_ENDS[1:]), flat, side="right"), 0, 3
    )
    local = flat - np.asarray(CUTOFF_ENDS)[cluster]

    unit_pos = {}
    for u in UNIT_KEYS:
        if u == 0 or u == 1 or u == 3:
            unit_pos[u] = np.nonzero(cluster == u)[0]
        else:
            r = u[1]
            unit_pos[u] = np.nonzero((cluster == 2) & (local // C2_SUB == r))[0]

    core_lists = {u: [unit_pos[u][k::N_CORES] for k in range(N_CORES)]
                  for u in UNIT_KEYS}
    Ks = {
        u: int(-(-max(len(core_lists[u][k]) for k in range(N_CORES)) // P))
        for u in UNIT_KEYS
    }
    G = sum(Ks.values())
    K3 = Ks[3]

    def idxval(u, positions):
        lv = local[positions]
        if u == 0 or u == 1:
            return lv.astype(np.int16)
        if u == 3:
            return (lv // C3_PACK).astype(np.int16)
        return (lv - u[1] * C2_SUB).astype(np.int16)

    NI = 8 * G
    gbase = {}
    acc = 0
    for u in UNIT_KEYS:
        gbase[u] = acc
        acc += Ks[u]

    NAUX = max(K3, 1) * 128
    blkid = np.arange(128) // 16  # sub-row block of each super-row element

    idx_maps, aux_maps, row_maps = [], [], []
    for k in range(N_CORES):
        cols = []
        row_map = np.full(G * P, -1, dtype=np.int64)
        aux = np.zeros((P, NAUX), dtype=np.float32)
        for u in UNIT_KEYS:
            n = Ks[u]
            if n == 0:
                continue
            lst = core_lists[u][k]
            fill = -1 if u in FUSED_UNITS else 0
            cols.append(_wrap_idx16(idxval(u, lst), n * P, fill=fill))
            m = np.arange(len(lst))
            row_map[(gbase[u] + m // P) * P + (m % P)] = lst
            if u == 3:
                # transposed mask layout: [dim-partition, token col]
                s_arr = local[lst] % C3_PACK
                mask = np.zeros((P, K3 * P), dtype=np.float32)
                mask[:, m] = (blkid[:, None] == s_arr[None, :])
                aux[:, 0:K3 * P] = mask
        idx_host = (np.concatenate(cols, axis=1) if cols
                    else np.zeros((P, 16), np.int16))
        if idx_host.shape[1] < max(NI, 16):
            pad = np.zeros((P, max(NI, 16) - idx_host.shape[1]), np.int16)
            idx_host = np.concatenate([idx_host, pad], axis=1)
        idx_maps.append(np.ascontiguousarray(idx_host))
        aux_maps.append(aux.astype(BF16))
        row_maps.append(row_map)

    # ---- table/projection prep -----------------------------------------
    # clusters 0/1 fused on host: table' = (emb @ proj.T) * scale, bf16
    fe0 = np.ascontiguousarray(
        (embs[0].astype(np.float32) @ projs[0].T.astype(np.float32)
         * EMB_SCALE).astype(BF16))
    fe1 = np.ascontiguousarray(
        (embs[1].astype(np.float32) @ projs[1].T.astype(np.float32)
         * EMB_SCALE).astype(BF16))
    emb2p = np.zeros((C2_SUB * C2_NSUB, 128), dtype=BF16)
    emb2p[:160000, :64] = embs[2].astype(BF16)
    e3flat = embs[3].astype(np.float32)
    pad3 = C3_SROWS * C3_PACK - e3flat.shape[0]
    e3flat = np.concatenate([e3flat, np.zeros((pad3, 16), np.float32)], axis=0)
    emb3p = np.ascontiguousarray(e3flat.reshape(C3_SROWS, 128).astype(BF16))

    pt2 = np.ascontiguousarray(
        (projs[2].T.astype(np.float32) * EMB_SCALE).astype(BF16)
        .reshape(1, 64, D_PROJ).transpose(1, 0, 2))
    pt3 = projs[3].T.astype(np.float32) * EMB_SCALE
    pt3s = np.ascontiguousarray(
        np.tile(pt3, (C3_PACK, 1)).astype(BF16).reshape(128, 1, D_PROJ)
    )

    in_maps = []
    for k in range(N_CORES):
        m = {
            "idx16": idx_maps[k], "aux": aux_maps[k],
            "fe0": fe0, "fe1": fe1, "emb2p": emb2p, "emb3p": emb3p,
            "pt2": pt2, "pt3s": pt3s,
        }
        in_maps.append(m)

    # ---- device --------------------------------------------------------
    rows_g = {}
    for u in UNIT_KEYS:
        maxcnt = max(len(core_lists[u][k]) for k in range(N_CORES))
        for t in range(Ks[u]):
            rows_g[gbase[u] + t] = int(min(P, max(1, maxcnt - t * P)))
    nc = _build_graph(Ks, rows_g)
    res = run_bass_kernel_spmd(
        nc,
        in_maps,
        core_ids=list(range(N_CORES)),
        trace=TRACE,
        trace_cores=TRACE_CORES,
    )
    LAST["res"] = res
    LAST["Ks"] = Ks

    # ---- host-side unshard ---------------------------------------------
    out_full = np.zeros((T, D_PROJ), dtype=np.float32)
    for k in range(N_CORES):
        o = np.asarray(res.results[k]["out"])
        rm = row_maps[k]
        valid = rm >= 0
        out_full[rm[valid]] = o[valid].astype(np.float32)
    return out_full.reshape(B, S, D_PROJ)


# revision 5
# speedup vs baseline: 1.0437x; 1.0437x over previous
"""Adaptive embedding lookup (nn.AdaptiveEmbedding) on 8 TRN2 NeuronCores.

Strategy (data-parallel over tokens, tables replicated, no collectives):

Host:
  - Clusters 0, 1 and 3 are FUSED on host: table' = (emb @ proj.T) * scale
    in bf16 [vocab, 1024].  On device those clusters are a pure dma_gather
    (transpose=False, token-on-partition layout) whose destination tile is
    DMA'd straight to the output rows — no projection load, no matmul.
    Cluster 3's fused table is split into 3 row sub-ranges of 32000 so
    dma_gather's int16 indices stay in range; same for cluster 2 (5 subs).
  - Cluster 2 (d=64, 60% of tokens) stays gather+matmul: fusing it would
    inflate its gather traffic 8x.  Its table is bf16 zero-padded to 256B
    rows (dma_gather granularity), projection pre-transposed, pre-scaled,
    bf16.
  - Tokens are dealt round-robin to the 8 cores per unit, padded to a
    multiple of 128 (one "group" of 128 output rows).  Fused-cluster pad
    indices are -1 (skipped by the gather ucode -> no wasted bandwidth);
    output DMAs are trimmed to the rows actually used.

Device (SPMD, identical graph on all 8 cores, one TileContext):
  - Cluster-2 gathers are emitted FIRST (gpsimd descriptor-gen is serial,
    ~0.7us per gather call) so the serial PE pipeline starts as early as
    possible; fused gathers follow and their gather->output DMAs are issued
    from the otherwise-idle gpsimd engine so they never block the cluster-2
    matmul/copy/output path on sync/scalar.
  - Per cluster-2 group: matmul lhsT=[64, 128] tokens against projection
    [64, 2x512] accumulating in PSUM, copy (f32->bf16 cast) to SBUF, DMA
    the trimmed [rows, 1024] output out.

Host: inverse-permute the 8 per-core outputs into [8, 2048, 1024] f32.
"""

import numpy as np
import ml_dtypes

import concourse.bacc as bacc
import concourse.bass as bass
import concourse.mybir as mybir
from concourse.bass_utils import run_bass_kernel_spmd
from concourse.tile import TileContext

N_TOKEN = 267735
D_PROJ = 1024
CUTOFF_ENDS = [0, 20000, 40000, 200000, 267735]
D_EMBS = [1024, 256, 64, 16]
EMB_SCALE = float(D_PROJ) ** 0.5
N_CORES = 8
P = 128
NFREE = 512          # psum free-dim per matmul
C2_SUB = 32000       # cluster-2 subtable rows (int16 range)
C2_NSUB = 5
C3_SUB = 32000       # cluster-3 fused subtable rows
C3_NSUB = 3          # 67735 -> 32000, 32000, 3735
N_C3 = CUTOFF_ENDS[4] - CUTOFF_ENDS[3]

BF16 = ml_dtypes.bfloat16

# Test-harness knobs (the grader never touches these).
TRACE = False
TRACE_CORES = None
LAST = {}

_GRAPH_CACHE = {}

# unit = gather bucket.  0, 1 fused; (2, r) matmul sub-ranges; (3, r) fused
# sub-ranges.
UNIT_KEYS = ([0, 1] + [(2, r) for r in range(C2_NSUB)]
             + [(3, r) for r in range(C3_NSUB)])


def _is_fused(u):
    return u in (0, 1) or (isinstance(u, tuple) and u[0] == 3)


def _build_graph(Ks, rows_g):
    """Ks: dict unit_key -> group count (0 allowed); rows_g: global group ->
    output rows actually used (<=128, pad rows trimmed from the out DMA).
    Same on all cores."""
    key = (tuple(Ks[u] for u in UNIT_KEYS), tuple(sorted(rows_g.items())))
    if key in _GRAPH_CACHE:
        return _GRAPH_CACHE[key]

    NI = 8 * sum(Ks.values())          # idx16 columns (8 per group)
    G = sum(Ks.values())               # total output groups

    nc = bacc.Bacc("TRN2", debug=False, num_swdge_queues=4)
    idx_ext = nc.declare_dram_parameter("idx16", [P, max(NI, 16)], mybir.dt.int16, False)
    fe0_ext = nc.declare_dram_parameter("fe0", [20000, D_PROJ], mybir.dt.bfloat16, False)
    fe1_ext = nc.declare_dram_parameter("fe1", [20000, D_PROJ], mybir.dt.bfloat16, False)
    fe3_ext = nc.declare_dram_parameter("fe3", [N_C3, D_PROJ], mybir.dt.bfloat16, False)
    emb2_ext = nc.declare_dram_parameter("emb2p", [C2_SUB * C2_NSUB, 128], mybir.dt.bfloat16, False)
    pt2_ext = nc.declare_dram_parameter("pt2", [64, 1, D_PROJ], mybir.dt.bfloat16, False)
    out_ext = nc.declare_dram_parameter("out", [G * P, D_PROJ], mybir.dt.bfloat16, True)

    with TileContext(nc) as tc:
        with tc.tile_pool(name="const", bufs=1) as constp, \
             tc.tile_pool(name="work", bufs=8) as workp, \
             tc.tile_pool(name="ps_o", bufs=8, space="PSUM") as psump:
            idx_sb = constp.tile([P, max(NI, 16)], mybir.dt.int16, tag="idx")
            nc.sync.dma_start(out=idx_sb[:], in_=idx_ext[:])
            pt2_sb = constp.tile([64, 1, D_PROJ], mybir.dt.bfloat16, tag="pt2")
            nc.sync.dma_start(out=pt2_sb[:], in_=pt2_ext[:])

            # fused-cluster gather destinations: token-on-partition layout,
            # gathered row m -> partition m%128, group m//128
            f_units = [u for u in UNIT_KEYS if _is_fused(u)]
            f_sb = {
                u: constp.tile([P, max(Ks[u], 1), D_PROJ], mybir.dt.bfloat16,
                               tag=f"f{i}", name=f"f{i}")
                for i, u in enumerate(f_units)
            }
            fe_ext = {0: fe0_ext[:], 1: fe1_ext[:]}
            for r in range(C3_NSUB):
                lo = r * C3_SUB
                fe_ext[(3, r)] = fe3_ext[lo:min(lo + C3_SUB, N_C3), :]
            # cluster-2 lhsT gather destinations ([dim, token] layout): the
            # c2 table is zero-padded to 128-elem bf16 rows so no on-device
            # casts or PE transposes are needed
            e2s = [
                constp.tile([P, 1, max(Ks[(2, r)], 1) * P], mybir.dt.bfloat16,
                            tag=f"e2_{r}", name=f"e2_{r}")
                for r in range(C2_NSUB)
            ]

            # idx16 column offset per unit (indices are packed in UNIT_KEYS order)
            unit_col = {}
            col = 0
            for u in UNIT_KEYS:
                unit_col[u] = col
                col += 8 * Ks[u]
            # global group index per unit
            gbase_dev = {}
            acc_g = 0
            for u in UNIT_KEYS:
                gbase_dev[u] = acc_g
                acc_g += Ks[u]

            # gather emission order: cluster-2 first (it feeds the serial PE
            # pipeline), then fused units; the first group of f0/f1 is split
            # out so its output DMA can start as soon as it lands
            gather_list = []  # (unit, group_lo, n_groups, transposed)
            for r in range(C2_NSUB):
                if Ks[(2, r)] > 0:
                    gather_list.append(((2, r), 0, Ks[(2, r)], True))
            fused_order = []
            for u in (0, 1):
                if Ks[u] > 0:
                    fused_order.append((u, 0, 1))
            for r in range(C3_NSUB):
                if Ks[(3, r)] > 0:
                    fused_order.append(((3, r), 0, Ks[(3, r)]))
            for u in (0, 1):
                for glo in range(1, Ks[u]):
                    fused_order.append((u, glo, 1))
            gather_list += [(u, glo, n, False) for (u, glo, n) in fused_order]

            for qi, (u, glo, n, trans) in enumerate(gather_list):
                c0 = unit_col[u] + 8 * glo
                if trans:
                    r = u[1]
                    nc.gpsimd.dma_gather(
                        e2s[r][:], emb2_ext[r * C2_SUB:(r + 1) * C2_SUB, :],
                        idx_sb[:, c0:c0 + 8 * n], n * P, n * P, 128,
                        transpose=True,
                        queue_num=qi % 4,
                    )
                else:
                    nc.gpsimd.dma_gather(
                        f_sb[u][:, glo:glo + n, :], fe_ext[u],
                        idx_sb[:, c0:c0 + 8 * n], n * P, n * P, D_PROJ,
                        transpose=False,
                        queue_num=qi % 4,
                    )

            # cluster-2 matmul groups, PE stream pinned in emission order
            # with no-sync scheduling edges
            import bass_rust as _br2
            last_pe_inst = [None]

            def emit_group(g, lhsT_of):
                rows = rows_g[g]
                osb = workp.tile([P, D_PROJ], mybir.dt.bfloat16, tag="osb")
                ps0 = psump.tile([P, NFREE], mybir.dt.float32, tag="ps")
                ps1 = psump.tile([P, NFREE], mybir.dt.float32, tag="ps")
                for oc, ps in enumerate((ps0, ps1)):
                    mm = nc.tensor.matmul(
                        out=ps[:],
                        lhsT=lhsT_of(),
                        rhs=pt2_sb[:64, 0, oc * NFREE:(oc + 1) * NFREE],
                        start=True,
                        stop=True,
                    )
                    if last_pe_inst[0] is not None:
                        _br2.add_dep_helper(
                            mm.ins, last_pe_inst[0], sync=False,
                            reason="pin PE stream order",
                        )
                    last_pe_inst[0] = mm.ins
                    nc.any.tensor_copy(
                        out=osb[:, oc * NFREE:(oc + 1) * NFREE], in_=ps[:]
                    )
                out_eng = nc.sync if g % 2 == 0 else nc.scalar
                out_eng.dma_start(
                    out=out_ext[g * P:g * P + rows, :], in_=osb[:rows, :]
                )

            jg = 0
            for r in range(C2_NSUB):
                for j in range(Ks[(2, r)]):
                    gg = gbase_dev[(2, 0)] + jg
                    emit_group(
                        gg,
                        lambda _r=r, _j=j: e2s[_r][:64, 0, _j * P:(_j + 1) * P],
                    )
                    jg += 1

            # fused clusters: gather dst -> trimmed output DMA, issued from
            # the gpsimd engine (idle once descriptor-gen is done) in gather
            # landing order
            for (u, glo, n) in fused_order:
                for j in range(glo, glo + n):
                    g = gbase_dev[u] + j
                    rows = rows_g[g]
                    nc.gpsimd.dma_start(
                        out=out_ext[g * P:g * P + rows, :],
                        in_=f_sb[u][:rows, j, :],
                    )

    nc.compile()
    _GRAPH_CACHE[key] = nc
    return nc


def _wrap_idx16(vals, n_slots, fill=0):
    """int16 values (len <= n_slots, padded with `fill`) -> [128, n_slots/16]
    wrapped."""
    full = np.full(n_slots, fill, dtype=np.int16)
    full[:len(vals)] = vals
    w = np.zeros((16, n_slots // 16), dtype=np.int16)
    m = np.arange(n_slots)
    w[m % 16, m // 16] = full
    return np.tile(w, (8, 1))


def kernel(inp, emb0, emb1, emb2, emb3, proj0, proj1, proj2, proj3):
    inp = np.asarray(inp)
    embs = [np.asarray(e) for e in (emb0, emb1, emb2, emb3)]
    projs = [np.asarray(p) for p in (proj0, proj1, proj2, proj3)]
    B, S = inp.shape
    flat = inp.reshape(-1).astype(np.int64)
    T = flat.shape[0]

    # ---- host-side bucketing -------------------------------------------
    flat = np.clip(flat, 0, N_TOKEN - 1)
    cluster = np.clip(
        np.searchsorted(np.asarray(CUTOFF_ENDS[1:]), flat, side="right"), 0, 3
    )
    local = flat - np.asarray(CUTOFF_ENDS)[cluster]

    unit_pos = {}
    for u in UNIT_KEYS:
        if u == 0 or u == 1:
            unit_pos[u] = np.nonzero(cluster == u)[0]
        elif u[0] == 2:
            unit_pos[u] = np.nonzero((cluster == 2) & (local // C2_SUB == u[1]))[0]
        else:
            unit_pos[u] = np.nonzero((cluster == 3) & (local // C3_SUB == u[1]))[0]

    core_lists = {u: [unit_pos[u][k::N_CORES] for k in range(N_CORES)]
                  for u in UNIT_KEYS}
    Ks = {
        u: int(-(-max(len(core_lists[u][k]) for k in range(N_CORES)) // P))
        for u in UNIT_KEYS
    }
    G = sum(Ks.values())

    def idxval(u, positions):
        lv = local[positions]
        if u == 0 or u == 1:
            return lv.astype(np.int16)
        return (lv - u[1] * (C2_SUB if u[0] == 2 else C3_SUB)).astype(np.int16)

    NI = 8 * G
    gbase = {}
    acc = 0
    for u in UNIT_KEYS:
        gbase[u] = acc
        acc += Ks[u]

    idx_maps, row_maps = [], []
    for k in range(N_CORES):
        cols = []
        row_map = np.full(G * P, -1, dtype=np.int64)
        for u in UNIT_KEYS:
            n = Ks[u]
            if n == 0:
                continue
            lst = core_lists[u][k]
            fill = -1 if _is_fused(u) else 0
            cols.append(_wrap_idx16(idxval(u, lst), n * P, fill=fill))
            m = np.arange(len(lst))
            row_map[(gbase[u] + m // P) * P + (m % P)] = lst
        idx_host = (np.concatenate(cols, axis=1) if cols
                    else np.zeros((P, 16), np.int16))
        if idx_host.shape[1] < max(NI, 16):
            pad = np.zeros((P, max(NI, 16) - idx_host.shape[1]), np.int16)
            idx_host = np.concatenate([idx_host, pad], axis=1)
        idx_maps.append(np.ascontiguousarray(idx_host))
        row_maps.append(row_map)

    # ---- table/projection prep -----------------------------------------
    # clusters 0/1/3 fused on host: table' = (emb @ proj.T) * scale, bf16
    def fuse(e, p):
        return np.ascontiguousarray(
            (e.astype(np.float32) @ p.T.astype(np.float32)
             * EMB_SCALE).astype(BF16))

    fe0 = fuse(embs[0], projs[0])
    fe1 = fuse(embs[1], projs[1])
    fe3 = fuse(embs[3], projs[3])
    emb2p = np.zeros((C2_SUB * C2_NSUB, 128), dtype=BF16)
    emb2p[:160000, :64] = embs[2].astype(BF16)
    pt2 = np.ascontiguousarray(
        (projs[2].T.astype(np.float32) * EMB_SCALE).astype(BF16)
        .reshape(1, 64, D_PROJ).transpose(1, 0, 2))

    in_maps = []
    for k in range(N_CORES):
        m = {
            "idx16": idx_maps[k],
            "fe0": fe0, "fe1": fe1, "fe3": fe3,
            "emb2p": emb2p, "pt2": pt2,
        }
        in_maps.append(m)

    # ---- device --------------------------------------------------------
    rows_g = {}
    for u in UNIT_KEYS:
        maxcnt = max(len(core_lists[u][k]) for k in range(N_CORES))
        for t in range(Ks[u]):
            rows_g[gbase[u] + t] = int(min(P, max(1, maxcnt - t * P)))
    nc = _build_graph(Ks, rows_g)
    res = run_bass_kernel_spmd(
        nc,
        in_maps,
        core_ids=list(range(N_CORES)),
        trace=TRACE,
        trace_cores=TRACE_CORES,
    )
    LAST["res"] = res
    LAST["Ks"] = Ks

    # ---- host-side unshard ---------------------------------------------
    out_full = np.zeros((T, D_PROJ), dtype=np.float32)
    for k in range(N_CORES):
        o = np.asarray(res.results[k]["out"])
        rm = row_maps[k]
        valid = rm >= 0
        out_full[rm[valid]] = o[valid].astype(np.float32)
    return out_full.reshape(B, S, D_PROJ)


# revision 15
# speedup vs baseline: 1.1969x; 1.1468x over previous
"""Adaptive embedding lookup (nn.AdaptiveEmbedding) on 8 TRN2 NeuronCores.

Strategy (data-parallel over tokens, tables replicated, no collectives):

Host:
  - Clusters 0, 1 and 3 are FUSED on host: table' = (emb @ proj.T) * scale
    in bf16 [vocab, 1024].  On device those clusters are a pure indirect
    gather (token-on-partition layout) whose destination tile is DMA'd
    straight to the output rows — no projection load, no matmul.
  - Cluster 2 (d=64, 60% of tokens) stays gather+matmul: fusing it would
    inflate its gather traffic 16x.  Its rows are gathered token-major
    (128 B each), PE-transposed on device into lhsT layout, then projected.
  - All gathers use gpsimd indirect_dma_start (plain SWDGE InstDMACopy with
    a dynamic access pattern): int32 indices — no 32k-row subtable split,
    no gather-ucode library reload (~10us), and ~10x cheaper descriptor
    generation than the transposed dma_gather path.
  - Tokens are dealt round-robin to the 8 cores per cluster, padded to a
    multiple of 128 (one "group" of 128 output rows).  Pad indices point
    past the bounds check and are silently skipped (no wasted bandwidth);
    output DMAs are trimmed to the rows actually used.

Device (SPMD, identical graph on all 8 cores, one TileContext):
  - gpsimd issues the indirect gathers: cluster-2 groups first (they feed
    the serial PE pipeline), interleaved with the fused-cluster gathers.
  - Per cluster-2 group: PE-transpose [128tok, 64] -> [64, 128tok], copy to
    SBUF, matmul against projection [64, 2x512] accumulating in PSUM, copy
    (f32->bf16 cast) to SBUF, DMA the trimmed [rows, 1024] output out.
  - Fused clusters: gather dst -> trimmed output DMA on sync/scalar HWDGE.

Host: inverse-permute the 8 per-core outputs into [8, 2048, 1024] f32.
"""

import numpy as np
import ml_dtypes

import concourse.bacc as bacc
import concourse.bass as bass
import concourse.mybir as mybir
from concourse.bass_utils import run_bass_kernel_spmd
from concourse.tile import TileContext

N_TOKEN = 267735
D_PROJ = 1024
CUTOFF_ENDS = [0, 20000, 40000, 200000, 267735]
EMB_SCALE = float(D_PROJ) ** 0.5
N_CORES = 8
P = 128
NFREE = 512          # psum free-dim per matmul

BF16 = ml_dtypes.bfloat16

# Test-harness knobs (the grader never touches these).
TRACE = False
TRACE_CORES = None
LAST = {}

_GRAPH_CACHE = {}

UNIT_KEYS = [0, 1, 2, 3]
FUSED_UNITS = (0, 1, 3)
VOCABS = {0: 20000, 1: 20000, 2: 160000, 3: 67735}


def _build_graph(Ks, rows_g):
    """Ks: dict unit -> group count (0 allowed); rows_g: global group ->
    output rows actually used (<=128, pad rows trimmed from the out DMA).
    Same on all cores."""
    key = (tuple(Ks[u] for u in UNIT_KEYS), tuple(sorted(rows_g.items())))
    if key in _GRAPH_CACHE:
        return _GRAPH_CACHE[key]

    G = sum(Ks.values())               # total output groups
    K2 = Ks[2]

    nc = bacc.Bacc("TRN2", debug=False, num_swdge_queues=4)
    idx_ext = nc.declare_dram_parameter("idx32", [P, max(G, 4)], mybir.dt.int32, False)
    fe_exts = {
        u: nc.declare_dram_parameter(f"fe{u}", [VOCABS[u], D_PROJ],
                                     mybir.dt.bfloat16, False)
        for u in FUSED_UNITS
    }
    emb2_ext = nc.declare_dram_parameter("emb2b", [VOCABS[2], 64], mybir.dt.bfloat16, False)
    pt2_ext = nc.declare_dram_parameter("pt2", [64, 1, D_PROJ], mybir.dt.bfloat16, False)
    id_ext = nc.declare_dram_parameter("ident", [P, P], mybir.dt.bfloat16, False)
    out_ext = nc.declare_dram_parameter("out", [G * P, D_PROJ], mybir.dt.bfloat16, True)

    with TileContext(nc) as tc:
        with tc.tile_pool(name="const", bufs=1) as constp, \
             tc.tile_pool(name="work", bufs=8) as workp, \
             tc.tile_pool(name="ps_mm", bufs=6, space="PSUM") as psump, \
             tc.tile_pool(name="ps_t", bufs=2, space="PSUM") as psumt:
            idx_sb = constp.tile([P, max(G, 4)], mybir.dt.int32, tag="idx")
            nc.sync.dma_start(out=idx_sb[:], in_=idx_ext[:])
            pt2_sb = constp.tile([64, 1, D_PROJ], mybir.dt.bfloat16, tag="pt2")
            nc.sync.dma_start(out=pt2_sb[:], in_=pt2_ext[:])
            id_sb = constp.tile([P, P], mybir.dt.bfloat16, tag="ident")
            nc.scalar.dma_start(out=id_sb[:], in_=id_ext[:])

            # fused-cluster gather destinations (token-on-partition: token
            # j*128+p of unit u lands at [p, j, :]); cluster-2 token-major
            # NOTE: gather destinations must be plain 2-dim [128, n]
            # slices — a size-1 middle dim breaks the HW dynamic-AP lowering
            f_sb = {
                u: constp.tile([P, max(Ks[u], 1) * D_PROJ], mybir.dt.bfloat16,
                               tag=f"f{u}", name=f"f{u}")
                for u in FUSED_UNITS
            }
            e2g = constp.tile([P, max(K2, 1) * 64], mybir.dt.bfloat16, tag="e2g")

            # global group index per unit (output-row blocks in UNIT_KEYS order)
            gbase_dev = {}
            acc_g = 0
            for u in UNIT_KEYS:
                gbase_dev[u] = acc_g
                acc_g += Ks[u]

            def gather(u, dst_tile, elem, dstc0, col0, ncols):
                # one call per group of 128 tokens: [128, 1] offsets, 2-dim
                # [128, elem] dst slice (pad indices gather row 0 harmlessly;
                # pad output rows are trimmed from the out DMA)
                tab = emb2_ext if u == 2 else fe_exts[u]
                for c in range(ncols):
                    nc.gpsimd.indirect_dma_start(
                        out=dst_tile[:, (dstc0 + c) * elem:(dstc0 + c + 1) * elem],
                        out_offset=None,
                        in_=tab[:],
                        in_offset=bass.IndirectOffsetOnAxis(
                            ap=idx_sb[:, col0 + c:col0 + c + 1], axis=0,
                        ),
                    )

            # PE stream pinned in emission order with no-sync scheduling edges
            import bass_rust as _br2
            last_pe_inst = [None]
            out_flip = [0]

            def pin(mm):
                if last_pe_inst[0] is not None:
                    _br2.add_dep_helper(
                        mm.ins, last_pe_inst[0], sync=False,
                        reason="pin PE stream order",
                    )
                last_pe_inst[0] = mm.ins

            def emit_c2_group(j):
                g = gbase_dev[2] + j
                rows = rows_g[g]
                tp = psumt.tile([64, P], mybir.dt.bfloat16, tag="tp")
                mm = nc.tensor.transpose(
                    out=tp[:], in_=e2g[:, j * 64:(j + 1) * 64], identity=id_sb[:]
                )
                pin(mm)
                lt = workp.tile([64, P], mybir.dt.bfloat16, tag="lt")
                nc.vector.tensor_copy(out=lt[:], in_=tp[:])
                osb = workp.tile([P, D_PROJ], mybir.dt.bfloat16, tag="osb")
                for oc in range(2):
                    ps = psump.tile([P, NFREE], mybir.dt.float32, tag="ps")
                    mm = nc.tensor.matmul(
                        out=ps[:],
                        lhsT=lt[:64, :],
                        rhs=pt2_sb[:64, 0, oc * NFREE:(oc + 1) * NFREE],
                        start=True,
                        stop=True,
                    )
                    pin(mm)
                    if oc == 0:
                        nc.vector.tensor_copy(
                            out=osb[:, oc * NFREE:(oc + 1) * NFREE], in_=ps[:]
                        )
                    else:
                        nc.scalar.copy(
                            out=osb[:, oc * NFREE:(oc + 1) * NFREE], in_=ps[:]
                        )
                out_eng = nc.sync if out_flip[0] % 2 == 0 else nc.scalar
                out_flip[0] += 1
                out_eng.dma_start(
                    out=out_ext[g * P:g * P + rows, :], in_=osb[:rows, :]
                )

            def emit_fused_outs(u):
                for j in range(Ks[u]):
                    g = gbase_dev[u] + j
                    rows = rows_g[g]
                    out_eng = nc.sync if out_flip[0] % 2 == 0 else nc.scalar
                    out_flip[0] += 1
                    out_eng.dma_start(
                        out=out_ext[g * P:g * P + rows, :],
                        in_=f_sb[u][:rows, j * D_PROJ:(j + 1) * D_PROJ],
                    )

            # interleaved emission: cluster-2 gathers/compute keep the PE
            # fed from the start; fused units slot in between so their
            # output DMAs flow while cluster 2 is still computing
            c2_batches = [range(0, min(4, K2)), range(4, min(7, K2)),
                          range(7, K2)]
            fused_after = {0: (0,), 1: (1,), 2: (3,)}
            for bi, batch in enumerate(c2_batches):
                for j in batch:
                    gather(2, e2g, 64, j, gbase_dev[2] + j, 1)
                for u in fused_after[bi]:
                    if Ks[u] > 0:
                        gather(u, f_sb[u], D_PROJ, 0, gbase_dev[u], Ks[u])
                for j in batch:
                    emit_c2_group(j)
                for u in fused_after[bi]:
                    if Ks[u] > 0:
                        emit_fused_outs(u)

    nc.compile()
    _GRAPH_CACHE[key] = nc
    return nc


def kernel(inp, emb0, emb1, emb2, emb3, proj0, proj1, proj2, proj3):
    inp = np.asarray(inp)
    embs = [np.asarray(e) for e in (emb0, emb1, emb2, emb3)]
    projs = [np.asarray(p) for p in (proj0, proj1, proj2, proj3)]
    B, S = inp.shape
    flat = inp.reshape(-1).astype(np.int64)
    T = flat.shape[0]

    # ---- host-side bucketing -------------------------------------------
    flat = np.clip(flat, 0, N_TOKEN - 1)
    cluster = np.clip(
        np.searchsorted(np.asarray(CUTOFF_ENDS[1:]), flat, side="right"), 0, 3
    )
    local = flat - np.asarray(CUTOFF_ENDS)[cluster]

    unit_pos = {u: np.nonzero(cluster == u)[0] for u in UNIT_KEYS}
    core_lists = {u: [unit_pos[u][k::N_CORES] for k in range(N_CORES)]
                  for u in UNIT_KEYS}
    Ks = {
        u: int(-(-max(len(core_lists[u][k]) for k in range(N_CORES)) // P))
        for u in UNIT_KEYS
    }
    G = sum(Ks.values())

    gbase = {}
    acc = 0
    for u in UNIT_KEYS:
        gbase[u] = acc
        acc += Ks[u]

    idx_maps, row_maps = [], []
    for k in range(N_CORES):
        # pad indices stay 0 (gather row 0 harmlessly; those output rows are
        # trimmed from the out DMA and dropped on host)
        idx_host = np.zeros((P, max(G, 4)), dtype=np.int32)
        row_map = np.full(G * P, -1, dtype=np.int64)
        for u in UNIT_KEYS:
            if Ks[u] == 0:
                continue
            lst = core_lists[u][k]
            m = np.arange(len(lst))
            idx_host[m % P, gbase[u] + m // P] = local[lst].astype(np.int32)
            row_map[(gbase[u] + m // P) * P + (m % P)] = lst
        idx_maps.append(np.ascontiguousarray(idx_host))
        row_maps.append(row_map)

    # ---- table/projection prep -----------------------------------------
    # clusters 0/1/3 fused on host: table' = (emb @ proj.T) * scale, bf16
    def fuse(e, p):
        return np.ascontiguousarray(
            (e.astype(np.float32) @ p.T.astype(np.float32)
             * EMB_SCALE).astype(BF16))

    fe = {0: fuse(embs[0], projs[0]), 1: fuse(embs[1], projs[1]),
          3: fuse(embs[3], projs[3])}
    emb2b = np.ascontiguousarray(embs[2].astype(BF16))
    pt2 = np.ascontiguousarray(
        (projs[2].T.astype(np.float32) * EMB_SCALE).astype(BF16)
        .reshape(1, 64, D_PROJ).transpose(1, 0, 2))
    ident = np.ascontiguousarray(np.eye(P, dtype=np.float32).astype(BF16))

    in_maps = []
    for k in range(N_CORES):
        m = {
            "idx32": idx_maps[k],
            "fe0": fe[0], "fe1": fe[1], "fe3": fe[3],
            "emb2b": emb2b, "pt2": pt2, "ident": ident,
        }
        in_maps.append(m)

    # ---- device --------------------------------------------------------
    rows_g = {}
    for u in UNIT_KEYS:
        maxcnt = max(len(core_lists[u][k]) for k in range(N_CORES))
        for t in range(Ks[u]):
            rows_g[gbase[u] + t] = int(min(P, max(1, maxcnt - t * P)))
    nc = _build_graph(Ks, rows_g)
    res = run_bass_kernel_spmd(
        nc,
        in_maps,
        core_ids=list(range(N_CORES)),
        trace=TRACE,
        trace_cores=TRACE_CORES,
    )
    LAST["res"] = res
    LAST["Ks"] = Ks

    # ---- host-side unshard ---------------------------------------------
    out_full = np.zeros((T, D_PROJ), dtype=np.float32)
    for k in range(N_CORES):
        o = np.asarray(res.results[k]["out"])
        rm = row_maps[k]
        valid = rm >= 0
        out_full[rm[valid]] = o[valid].astype(np.float32)
    return out_full.reshape(B, S, D_PROJ)
